# revision 6
# baseline (speedup 1.0000x reference)
"""LIF (leaky integrate-and-fire) recurrence kernel for Trainium2, 8 NeuronCores.

Problem: x (T=32, B=64, N=32768) f32.
    m[t] = tau*v[t-1] + x[t];  y[t] = (m[t] >= 1.0);  v[t] = m[t]*(1-y[t])
Output: y (32, 64, 32768) f32.

Sharding: data-parallel over batch. Core c handles x[:, 8c:8(c+1), :],
a (32, 262144)-element independent recurrence laid out [128, 2048] per step.

The serial per-element chain needs two dependent ALU ops per step; a single
engine (DVE) doing all of it is the 147us floor the previous revision sat
at. This version splits the 2048 columns into two lanes so four engines
share the recurrence:

Lane P (columns 0..1023, two 512-col groups), PSUM/TensorE pipeline in a
power-of-2 scaled domain M[t] = 2^t * m[t] (host pre-scales X[t] = 2^t*x[t],
exact): the decay tau=0.5 becomes a no-op, so
    M[t] = V[t-1] + X[t]     two accumulating fp32 identity matmuls into
                             PSUM (bit-exact: I-weights multiply by 1/0)
    s    = Sign(M - 2^t)     ACT, PSUM-sourced, {-1,0,+1}
    y    = Sigmoid(1e4*s+5e3)-> uint8   ACT (exact saturation; s=0 -> 1)
    V[t] = (s < 0) * M       one DVE STT reading s (SBUF port0) + M (PSUM
                             port) - leaves the shared SBUF port1 free
Lane D (columns 1024..2047), classic DVE pipeline in the unscaled domain:
    m = (v * tau) + x        DVE STT
    v = (m < 1) * m          DVE STT
    y = (m >= 1) -> uint8    GPSIMD tensor_scalar (GPSIMD shares SBUF port1
                             with 2-port DVE ops, but has huge slack)

y is stored as uint8 (0/1 exact; host widens to f32). t=0 skips the m/M
ops (v0=0, 2^0=1 so m0 = x0); t=31 skips the v/V ops (never consumed).
A dummy Sign/Sigmoid at the top preloads the ACT spline tables so the
1.3us table load hides under the first x DMA. x loads are staged
[1,3,4,...] on the sync HWDGE ring; y stores flush every 2 steps on the
scalar ring.
"""

import sys

if "/opt/trn_rl_repo" not in sys.path:
    sys.path.insert(0, "/opt/trn_rl_repo")

import numpy as np

TAU = 0.5
V_TH = 1.0

N_CORES = 8
T, B, N = 32, 64, 32768
B_SH = B // N_CORES          # 8 batch rows per core
E = B_SH * N                 # 262144 elements per core per timestep
P = 128                      # SBUF partitions
F = E // P                   # 2048 f32 per partition per timestep

G = 512                      # lane-P matmul group width (one PSUM bank)
N_G = 2                      # lane-P groups
FP = G * N_G                 # lane-P columns (PE/PSUM pipeline)
FD = F - FP                  # lane-D columns (DVE pipeline)

X_CHUNKS = [1, 3] + [4] * 7            # timesteps per x load (fast fill)
Y_CHUNKS = [2] * 15 + [1, 1]           # timesteps per y store (fast drain)

_compiled = None


def _build():
    from concourse import bacc, tile, mybir

    f32 = mybir.dt.float32
    ydt = mybir.dt.uint8
    Sign = mybir.ActivationFunctionType.Sign
    Sigmoid = mybir.ActivationFunctionType.Sigmoid
    assert sum(X_CHUNKS) == T and sum(Y_CHUNKS) == T
    nc = bacc.Bacc("TRN2", debug=False, num_devices=N_CORES)
    x = nc.dram_tensor("x", [T, E], f32, kind="ExternalInput").ap()
    ident = nc.dram_tensor("ident", [P, P], f32, kind="ExternalInput").ap()
    y = nc.dram_tensor("y", [T, E], ydt, kind="ExternalOutput").ap()

    # [t, p, f] views of DRAM
    x_r = x.rearrange("t (p f) -> t p f", p=P)
    y_r = y.rearrange("t (p f) -> t p f", p=P)

    with tile.TileContext(nc) as tc:
        with (
            tc.tile_pool(name="io", bufs=3) as io_pool,
            tc.tile_pool(name="state", bufs=1) as st_pool,
            tc.tile_pool(name="m", bufs=5) as m_pool,
            tc.tile_pool(name="s", bufs=4) as s_pool,
            tc.tile_pool(name="yp", bufs=3) as y_pool,
            tc.tile_pool(name="ps", bufs=3, space="PSUM") as ps_pool,
        ):
            # per-partition constants for the ACT affine args
            c_neg1 = st_pool.tile([P, 1], f32, tag="c_neg1")
            c_scale = st_pool.tile([P, 1], f32, tag="c_scale")
            c_bias = st_pool.tile([P, 1], f32, tag="c_bias")
            nc.gpsimd.memset(c_neg1[:], -V_TH)
            nc.gpsimd.memset(c_scale[:], 1.0e4)
            nc.gpsimd.memset(c_bias[:], 5.0e3)
            # lane-P thresholds: column t holds -2^t
            c_th = st_pool.tile([P, T], f32, tag="c_th")
            for t in range(T):
                nc.gpsimd.memset(c_th[:, t:t + 1], -float(2.0 ** t))
            # identity weights for the lane-P matmuls
            id_sb = st_pool.tile([P, P], f32, tag="ident")
            nc.sync.dma_start(out=id_sb[:], in_=ident)

            # warm the ACT spline tables while the first x chunk loads
            warm = st_pool.tile([P, 1], f32, tag="warm")
            nc.scalar.activation(out=warm[:], in_=c_neg1[:], func=Sign,
                                 bias=c_neg1[:], scale=1.0)
            nc.scalar.activation(out=warm[:], in_=warm[:], func=Sigmoid,
                                 bias=c_bias[:], scale=c_scale[:])

            # lane state
            v_d = st_pool.tile([P, FD], f32, tag="v_d")
            vp = [
                st_pool.tile([P, G], f32, name=f"v_p{g}", tag=f"v_p{g}")
                for g in range(N_G)
            ]

            x_tiles = {}          # t -> (tile, col offset)
            next_chunk = 0
            t_loaded = 0

            def load_chunk():
                nonlocal next_chunk, t_loaded
                n_t = X_CHUNKS[next_chunk]
                xt = io_pool.tile([P, 4 * F], f32, tag="x")
                nc.sync.dma_start(
                    out=xt[:, : n_t * F].rearrange("p (t f) -> p t f", t=n_t),
                    in_=x_r[t_loaded:t_loaded + n_t].rearrange("t p f -> p t f"),
                )
                for i in range(n_t):
                    x_tiles[t_loaded + i] = (xt, i * F)
                next_chunk += 1
                t_loaded += n_t

            load_chunk()
            y_t = None
            y_chunk_idx = 0
            y_off = 0  # timesteps into current y chunk
            for t in range(T):
                if t not in x_tiles:
                    load_chunk()
                if next_chunk < len(X_CHUNKS) and t == t_loaded - X_CHUNKS[next_chunk - 1]:
                    load_chunk()  # prefetch one chunk ahead
                xt, off = x_tiles.pop(t)
                n_yt = Y_CHUNKS[y_chunk_idx]
                if y_off == 0:
                    y_t = y_pool.tile([P, max(Y_CHUNKS) * F], ydt, tag="y")
                ys = y_t[:, y_off * F:(y_off + 1) * F]

                # ---- lane P: M = V + X via identity matmuls in PSUM ----
                for g in range(N_G):
                    xg = xt[:, off + g * G: off + (g + 1) * G]
                    pg = ps_pool.tile([P, G], f32, tag=f"ps{g}")
                    if t == 0:
                        nc.tensor.matmul(out=pg[:], lhsT=id_sb[:], rhs=xg,
                                         start=True, stop=True)
                    else:
                        nc.tensor.matmul(out=pg[:], lhsT=id_sb[:], rhs=vp[g][:],
                                         start=True, stop=False)
                        nc.tensor.matmul(out=pg[:], lhsT=id_sb[:], rhs=xg,
                                         start=False, stop=True)
                    s = s_pool.tile([P, G], f32, tag=f"s{g}")
                    nc.scalar.activation(out=s[:], in_=pg[:], func=Sign,
                                         bias=c_th[:, t:t + 1], scale=1.0)
                    nc.scalar.activation(out=ys[:, g * G:(g + 1) * G], in_=s[:],
                                         func=Sigmoid,
                                         bias=c_bias[:], scale=c_scale[:])
                    if t < T - 1:
                        # V = (s < 0) * M  (hard reset in the scaled domain)
                        nc.vector.scalar_tensor_tensor(
                            out=vp[g][:], in0=s[:], scalar=0.0, in1=pg[:],
                            op0=mybir.AluOpType.is_lt, op1=mybir.AluOpType.mult,
                        )

                # ---- lane D: classic DVE recurrence, GPSIMD spike ----
                xd = xt[:, off + FP: off + F]
                if t == 0:
                    md = xd
                else:
                    mt = m_pool.tile([P, FD], f32, tag="m")
                    nc.vector.scalar_tensor_tensor(
                        out=mt[:], in0=v_d[:], scalar=TAU, in1=xd,
                        op0=mybir.AluOpType.mult, op1=mybir.AluOpType.add,
                    )
                    md = mt[:]
                if t < T - 1:
                    nc.vector.scalar_tensor_tensor(
                        out=v_d[:], in0=md, scalar=V_TH, in1=md,
                        op0=mybir.AluOpType.is_lt, op1=mybir.AluOpType.mult,
                    )
                nc.gpsimd.tensor_scalar(
                    out=ys[:, FP:F], in0=md, scalar1=V_TH, scalar2=1.0,
                    op0=mybir.AluOpType.is_ge, op1=mybir.AluOpType.mult,
                )

                y_off += 1
                if y_off == n_yt:
                    nc.scalar.dma_start(
                        out=y_r[t - n_yt + 1:t + 1].rearrange("t p f -> p t f"),
                        in_=y_t[:, : n_yt * F].rearrange("p (t f) -> p t f", t=n_yt),
                    )
                    y_chunk_idx += 1
                    y_off = 0
    nc.compile()
    return nc


def _get_compiled():
    global _compiled
    if _compiled is None:
        _compiled = _build()
        # warm the NEFF (first execution pays ~20us of cold-start)
        import concourse.bass_utils as bass_utils

        z = [
            {"x": np.zeros((T, E), dtype=np.float32),
             "ident": np.eye(P, dtype=np.float32)}
            for _ in range(N_CORES)
        ]
        bass_utils.run_bass_kernel_spmd(
            _compiled, z, core_ids=list(range(N_CORES))
        )
    return _compiled


def kernel(x: np.ndarray, _trace: bool = False):
    import concourse.bass_utils as bass_utils

    nc = _get_compiled()
    x = np.ascontiguousarray(x, dtype=np.float32)
    # lane-P scaled domain: X[t] = 2^t * x[t] on columns f%2048 < FP
    # (exact power-of-2 scaling; lane D stays unscaled)
    xs = x.reshape(T, B, N // F, F).copy()
    pow2 = (2.0 ** np.arange(T)).astype(np.float32)
    xs[:, :, :, :FP] *= pow2[:, None, None, None]
    xs = xs.reshape(T, B, N)
    ident = np.eye(P, dtype=np.float32)
    in_maps = [
        {"x": xs[:, c * B_SH:(c + 1) * B_SH, :].reshape(T, E), "ident": ident}
        for c in range(N_CORES)
    ]
    res = bass_utils.run_bass_kernel_spmd(
        nc, in_maps, core_ids=list(range(N_CORES)), trace=_trace
    )
    y = np.empty((T, B, N), dtype=np.float32)
    for c in range(N_CORES):
        yc = res.results[c]["y"]
        if yc.dtype != np.float32:
            yc = yc.astype(np.float32)  # uint8 0/1 -> f32, exact
        y[:, c * B_SH:(c + 1) * B_SH, :] = yc.reshape(T, B_SH, N)
    if _trace:
        return y, res
    return y


# revision 7
# speedup vs baseline: 3.2792x; 3.2792x over previous
"""LIF (leaky integrate-and-fire) recurrence kernel for Trainium2, 8 NeuronCores.

Problem: x (T=32, B=64, N=32768) f32.
    m[t] = tau*v[t-1] + x[t];  y[t] = (m[t] >= 1.0);  v[t] = m[t]*(1-y[t])
Output: y (32, 64, 32768) f32.

Sharding: data-parallel over batch. Core c handles x[:, 8c:8(c+1), :],
a (32, 262144)-element independent recurrence laid out [128, 2048] per step.

Per-core pipeline (bit-exact vs the f32 reference):
  DVE (two fused scalar_tensor_tensor ops per step -- the serial chain and
  the bottleneck; ~2.29us per [128,2048] op):
    m = (v * tau) + x_t            (in0 op0 scalar) op1 in1
    v = (m is_lt 1.0) * m          hard reset: v=m below threshold, else 0
  ACT (spike output, exact at the threshold):
    s = Sign(m - 1)                m-1 is exact (Sterbenz), s in {-1,0,+1}
    y = Sigmoid(1e4*s + 5e3)       saturates: s=-1 -> 0.0, s in {0,+1} -> 1.0

y is stored as uint8 (0/1 exact; host widens to f32), quartering write
traffic vs f32. The t=0 m-op is skipped (v0=0 so m0 = x0, the x tile is
consumed directly); the t=31 v-op is skipped (v[31] unused). A dummy
Sign/Sigmoid pair at the top preloads the ACT spline tables so the ~2.6us
of table loads hide under the first x DMA instead of delaying the first
spike. x loads are staged [1,3,4,...] timesteps on the sync HWDGE ring;
y stores flush every 2 timesteps on the scalar ring (the ACT sequencer has
slack for the 667ns DMA configs; putting them on the sync ring would block
x-load configs behind store semaphores).

Rejected after measurement: GPSIMD spike offload (shares an SBUF port with
2-port DVE ops; its tensor_scalar degrades 12x next to a busy DVE) and a
TensorE identity-matmul lane for m (fp32 matmul = 2 passes + per-pass
LDWEIGHTS = ~3.5ns/col/step, more than the DVE op it replaces).
"""

import sys

if "/opt/trn_rl_repo" not in sys.path:
    sys.path.insert(0, "/opt/trn_rl_repo")

import numpy as np

TAU = 0.5
V_TH = 1.0

N_CORES = 8
T, B, N = 32, 64, 32768
B_SH = B // N_CORES          # 8 batch rows per core
E = B_SH * N                 # 262144 elements per core per timestep
P = 128                      # SBUF partitions
F = E // P                   # 2048 f32 per partition per timestep

X_CHUNKS = [1, 3] + [4] * 7            # timesteps per x load (fast fill)
Y_CHUNKS = [2] * 15 + [1, 1]           # timesteps per y store (short drain)
LAST_DVE_STEPS = 1                     # spike on DVE for the last step(s)

_compiled = None


def _build():
    from concourse import bacc, tile, mybir

    f32 = mybir.dt.float32
    ydt = mybir.dt.uint8
    Sign = mybir.ActivationFunctionType.Sign
    Sigmoid = mybir.ActivationFunctionType.Sigmoid
    assert sum(X_CHUNKS) == T and sum(Y_CHUNKS) == T
    nc = bacc.Bacc("TRN2", debug=False, num_devices=N_CORES)
    x = nc.dram_tensor("x", [T, E], f32, kind="ExternalInput").ap()
    y = nc.dram_tensor("y", [T, E], ydt, kind="ExternalOutput").ap()

    # [t, p, f] views of DRAM
    x_r = x.rearrange("t (p f) -> t p f", p=P)
    y_r = y.rearrange("t (p f) -> t p f", p=P)

    with tile.TileContext(nc) as tc:
        with (
            tc.tile_pool(name="io", bufs=3) as io_pool,
            tc.tile_pool(name="state", bufs=1) as st_pool,
            tc.tile_pool(name="m", bufs=5) as m_pool,
            tc.tile_pool(name="s", bufs=3) as s_pool,
            tc.tile_pool(name="yp", bufs=3) as y_pool,
        ):
            # per-partition constants for the ACT affine args
            c_neg1 = st_pool.tile([P, 1], f32, tag="c_neg1")
            c_scale = st_pool.tile([P, 1], f32, tag="c_scale")
            c_bias = st_pool.tile([P, 1], f32, tag="c_bias")
            nc.gpsimd.memset(c_neg1[:], -V_TH)
            nc.gpsimd.memset(c_scale[:], 1.0e4)
            nc.gpsimd.memset(c_bias[:], 5.0e3)
            v = st_pool.tile([P, F], f32, tag="v")

            # warm the ACT spline tables while the first x chunk loads
            warm = st_pool.tile([P, 1], f32, tag="warm")
            nc.scalar.activation(out=warm[:], in_=c_neg1[:], func=Sign,
                                 bias=c_neg1[:], scale=1.0)
            nc.scalar.activation(out=warm[:], in_=warm[:], func=Sigmoid,
                                 bias=c_bias[:], scale=c_scale[:])

            # issue x loads lazily, two chunks ahead of consumption
            x_tiles = {}          # t -> (tile, col offset)
            next_chunk = 0
            t_loaded = 0

            def load_chunk():
                nonlocal next_chunk, t_loaded
                n_t = X_CHUNKS[next_chunk]
                xt = io_pool.tile([P, 4 * F], f32, tag="x")
                nc.sync.dma_start(
                    out=xt[:, : n_t * F].rearrange("p (t f) -> p t f", t=n_t),
                    in_=x_r[t_loaded:t_loaded + n_t].rearrange("t p f -> p t f"),
                )
                for i in range(n_t):
                    x_tiles[t_loaded + i] = (xt, i * F)
                next_chunk += 1
                t_loaded += n_t

            load_chunk()
            y_t = None
            y_chunk_idx = 0
            y_off = 0  # timesteps into current y chunk
            for t in range(T):
                if t not in x_tiles:
                    load_chunk()
                if next_chunk < len(X_CHUNKS) and t == t_loaded - X_CHUNKS[next_chunk - 1]:
                    load_chunk()  # prefetch one chunk ahead
                xt, off = x_tiles.pop(t)
                xs = xt[:, off:off + F]
                n_yt = Y_CHUNKS[y_chunk_idx]
                if y_off == 0:
                    y_t = y_pool.tile([P, max(Y_CHUNKS) * F], ydt, tag="y")
                ys = y_t[:, y_off * F:(y_off + 1) * F]
                if t == 0:
                    # v0 = 0 so m0 = x0: consume the x tile as m directly
                    m = xs
                else:
                    mt = m_pool.tile([P, F], f32, tag="m")
                    # m = (v * tau) + x_t
                    nc.vector.scalar_tensor_tensor(
                        out=mt[:], in0=v[:], scalar=TAU, in1=xs,
                        op0=mybir.AluOpType.mult, op1=mybir.AluOpType.add,
                    )
                    m = mt[:]
                if t < T - 1:
                    # v = (m < vth) * m   (hard reset); v[31] is never used
                    nc.vector.scalar_tensor_tensor(
                        out=v[:], in0=m, scalar=V_TH, in1=m,
                        op0=mybir.AluOpType.is_lt, op1=mybir.AluOpType.mult,
                    )
                if t >= T - LAST_DVE_STEPS:
                    # tail steps: spike on DVE (fast 2x tensor_scalar) so the
                    # final stores don't wait for the ACT chain
                    nc.vector.tensor_scalar(
                        out=ys, in0=m, scalar1=V_TH, scalar2=1.0,
                        op0=mybir.AluOpType.is_ge, op1=mybir.AluOpType.mult,
                    )
                else:
                    # s = Sign(m - 1); y = Sigmoid(1e4*s + 5e3)
                    s = s_pool.tile([P, F], f32, tag="s")
                    nc.scalar.activation(
                        out=s[:], in_=m, func=Sign,
                        bias=c_neg1[:], scale=1.0,
                    )
                    nc.scalar.activation(
                        out=ys, in_=s[:], func=Sigmoid,
                        bias=c_bias[:], scale=c_scale[:],
                    )
                y_off += 1
                if y_off == n_yt:
                    nc.scalar.dma_start(
                        out=y_r[t - n_yt + 1:t + 1].rearrange("t p f -> p t f"),
                        in_=y_t[:, : n_yt * F].rearrange("p (t f) -> p t f", t=n_yt),
                    )
                    y_chunk_idx += 1
                    y_off = 0
    nc.compile()
    return nc


def _get_compiled():
    global _compiled
    if _compiled is None:
        _compiled = _build()
        # warm the NEFF (first execution pays ~20us of cold-start)
        import concourse.bass_utils as bass_utils

        z = [{"x": np.zeros((T, E), dtype=np.float32)} for _ in range(N_CORES)]
        bass_utils.run_bass_kernel_spmd(
            _compiled, z, core_ids=list(range(N_CORES))
        )
    return _compiled


def kernel(x: np.ndarray, _trace: bool = False):
    import concourse.bass_utils as bass_utils

    nc = _get_compiled()
    x = np.ascontiguousarray(x, dtype=np.float32)
    in_maps = [
        {"x": x[:, c * B_SH:(c + 1) * B_SH, :].reshape(T, E)}
        for c in range(N_CORES)
    ]
    res = bass_utils.run_bass_kernel_spmd(
        nc, in_maps, core_ids=list(range(N_CORES)), trace=_trace
    )
    y = np.empty((T, B, N), dtype=np.float32)
    for c in range(N_CORES):
        yc = res.results[c]["y"]
        if yc.dtype != np.float32:
            yc = yc.astype(np.float32)  # uint8 0/1 -> f32, exact
        y[:, c * B_SH:(c + 1) * B_SH, :] = yc.reshape(T, B_SH, N)
    if _trace:
        return y, res
    return y
